# revision 4
# baseline (speedup 1.0000x reference)
"""GCNConv (COO SpMM aggregation + dense GEMM) on 8 Trainium2 NeuronCores.

  msgs = edge_vals[:, None] * x[edge_col]          # [E, 64] gather+scale
  agg  = segment_sum(msgs, edge_row, N)            # [N, 64] scatter-add
  out  = agg @ weight                              # [N, 64] GEMM

Sharding: destination-node sharding (each core owns a contiguous row range
and all edges targeting it) -> zero collectives.  Host-side index prep sorts
edges by (core, 128-row dest block, x-quarter) and pads each (block, quarter)
group to a multiple of 128 edges.

Per core, per 128-edge chunk:
  - gpsimd.dma_gather pulls the 64-float source rows (256B each) from HBM
  - one DVE tensor_scalar builds vh[e, r] = (dest[e] == r) * val[e]
  - TensorE: psum[128 rows, 64] += vh.T @ msgs   (segment-sum as matmul)
Per 128-row block: PE-transpose agg -> aggT, then outT[64,128] = W.T @ aggT.
One contiguous [64, rows] output DMA per core; host concatenates+transposes.
"""

import os
import sys

import numpy as np

if "/opt/trn_rl_repo" not in sys.path:
    sys.path.insert(0, "/opt/trn_rl_repo")

# ---------------------------------------------------------------- constants
N = 100000
E = 1600000
D = 64
CORES = 8
RPC = 12544          # rows per core (= BLOCKS * 128; 8*12544 = 100352 >= N)
BLOCKS = RPC // 128  # 98 dest blocks per core
Q = 4                # x row-table quarters (int16 gather index limit)
XQ = 25088           # rows per quarter (4*25088 = 100352)
G = 7                # dest blocks per gather super-group (98 = 14*7)
NGROUPS = BLOCKS // G

LAST_EXEC_TIME_NS = None
_CACHE = {}


# ---------------------------------------------------------------- host prep
def _prep(x, weight, edge_vals, edge_row, edge_col):
    """Sort/pad edges; build per-core gather-index / dest / val arrays."""
    e_row = np.asarray(edge_row, dtype=np.int64)
    e_col = np.asarray(edge_col, dtype=np.int64)
    ev = np.asarray(edge_vals, dtype=np.float32)
    x = np.asarray(x, dtype=np.float32)
    weight = np.asarray(weight, dtype=np.float32)
    ne = e_row.shape[0]

    core = e_row // RPC
    blk = (e_row % RPC) // 128
    dest = (e_row % 128).astype(np.float32)
    qq = e_col // XQ
    lidx = (e_col - qq * XQ).astype(np.int16)

    NG = CORES * BLOCKS * Q
    gkey = (core * BLOCKS + blk) * Q + qq
    order = np.argsort(gkey, kind="stable")
    counts = np.bincount(gkey, minlength=NG)
    Cq = max(1, int(-(-counts.max() // 128)))  # chunks per (block, quarter)
    SL = Cq * 128
    starts = np.zeros(NG, np.int64)
    starts[1:] = np.cumsum(counts)[:-1]
    gs = gkey[order]
    slot = gs * SL + (np.arange(ne, dtype=np.int64) - starts[gs])

    idx_flat = np.zeros(NG * SL, np.int16)       # pad gathers row 0 (harmless)
    dst_flat = np.full(NG * SL, -1.0, np.float32)  # pad never matches iota
    val_flat = np.zeros(NG * SL, np.float32)       # pad scales to 0
    idx_flat[slot] = lidx[order]
    dst_flat[slot] = dest[order]
    val_flat[slot] = ev[order]

    CALLE = G * SL                   # edges per dma_gather call
    S16 = CALLE // 16

    def to_calls(a):
        # [CORES, BLOCKS, Q, SL] -> [CORES, NGROUPS, Q, G*SL] call-major
        a = a.reshape(CORES, NGROUPS, G, Q, SL)
        return np.ascontiguousarray(a.transpose(0, 1, 3, 2, 4)).reshape(
            CORES, NGROUPS, Q, CALLE
        )

    idx_c = to_calls(idx_flat)
    dst_c = to_calls(dst_flat)
    val_c = to_calls(val_flat)

    # gather idx wrap: idx for call-edge i lives at [i % 16, i // 16],
    # replicated across the 8 gpsimd cores -> [128, Q, CALLE//16]
    w16 = idx_c.reshape(CORES, NGROUPS, Q, S16, 16)
    w16 = np.moveaxis(w16, 4, 2)                      # [C, NGR, 16, Q, S16]
    gidx = np.ascontiguousarray(np.tile(w16, (1, 1, 8, 1, 1)))

    # dest/val layout matches gather output: [p, q, j] = call-edge j*128+p
    def to_pj(a):
        a = a.reshape(CORES, NGROUPS, Q, G * Cq, 128)
        return np.ascontiguousarray(np.moveaxis(a, 4, 2))  # [C,NGR,128,Q,G*Cq]

    gdst = to_pj(dst_c)
    gval = to_pj(val_c)

    x_pad = np.zeros((Q * XQ, D), np.float32)
    x_pad[:N] = x
    iota = np.tile(np.arange(128, dtype=np.float32), (128, 1))
    ident = np.eye(128, dtype=np.float32)

    in_maps = []
    for k in range(CORES):
        in_maps.append(
            {
                "xq": x_pad,
                "w": np.ascontiguousarray(weight),
                "iota": iota,
                "ident": ident,
                "gidx": gidx[k],
                "gdst": gdst[k],
                "gval": gval[k],
            }
        )
    return in_maps, Cq


# ------------------------------------------------------------- bass program
def _build(Cq):
    import concourse.bacc as bacc
    import concourse.mybir as mybir
    import concourse.tile as tile

    f32 = mybir.dt.float32
    i16 = mybir.dt.int16
    SL = Cq * 128
    CALLE = G * SL
    S16 = CALLE // 16
    JQ = G * Cq          # msgs slots per (call, quarter)

    nc = bacc.Bacc(
        "TRN2", target_bir_lowering=False, debug=False, num_devices=CORES
    )
    x_d = nc.dram_tensor("xq", [Q * XQ, D], f32, kind="ExternalInput")
    w_d = nc.dram_tensor("w", [D, D], f32, kind="ExternalInput")
    iota_d = nc.dram_tensor("iota", [128, 128], f32, kind="ExternalInput")
    id_d = nc.dram_tensor("ident", [128, 128], f32, kind="ExternalInput")
    gidx_d = nc.dram_tensor("gidx", [NGROUPS, 128, Q, S16], i16, kind="ExternalInput")
    gdst_d = nc.dram_tensor("gdst", [NGROUPS, 128, Q, JQ], f32, kind="ExternalInput")
    gval_d = nc.dram_tensor("gval", [NGROUPS, 128, Q, JQ], f32, kind="ExternalInput")
    outT_d = nc.dram_tensor("outT", [D, RPC], f32, kind="ExternalOutput")

    eq = mybir.AluOpType.is_equal
    mul = mybir.AluOpType.mult

    with tile.TileContext(nc) as tc:
        with (
            tc.tile_pool(name="const", bufs=1) as cpool,
            tc.tile_pool(name="io", bufs=2) as iopool,
            tc.tile_pool(name="msgs", bufs=2) as mpool,
            tc.tile_pool(name="vh", bufs=8) as vhpool,
            tc.tile_pool(name="sb", bufs=4) as sbpool,
            tc.tile_pool(name="outsb", bufs=1) as opool,
            tc.tile_pool(name="pa", bufs=3, space="PSUM") as papool,
            tc.tile_pool(name="pt", bufs=2, space="PSUM") as ptpool,
            tc.tile_pool(name="po", bufs=2, space="PSUM") as popool,
        ):
            w_sb = cpool.tile([D, D], f32, name="w_sb")
            iota_sb = cpool.tile([128, 128], f32, name="iota_sb")
            id_sb = cpool.tile([128, 128], f32, name="id_sb")
            outT_sb = opool.tile([D, RPC], f32, name="outT_sb")
            nc.sync.dma_start(out=w_sb[:], in_=w_d[:])
            nc.sync.dma_start(out=iota_sb[:], in_=iota_d[:])
            nc.sync.dma_start(out=id_sb[:], in_=id_d[:])

            for g in range(NGROUPS):
                idx_t = iopool.tile([128, Q, S16], i16, tag="idx", name=f"idx{g}")
                dst_t = iopool.tile([128, Q, JQ], f32, tag="dst", name=f"dst{g}")
                val_t = iopool.tile([128, Q, JQ], f32, tag="val", name=f"val{g}")
                nc.sync.dma_start(out=idx_t[:], in_=gidx_d[g])
                nc.sync.dma_start(out=dst_t[:], in_=gdst_d[g])
                nc.sync.dma_start(out=val_t[:], in_=gval_d[g])

                msgs = []
                for q in range(Q):
                    m = mpool.tile([128, JQ, D], f32, tag=f"msgs{q}", name=f"m{g}_{q}")
                    nc.gpsimd.dma_gather(
                        m[:],
                        x_d[q * XQ : (q + 1) * XQ, :],
                        idx_t[:, q, :],
                        CALLE,
                        CALLE,
                        D,
                        # default single_packet=True needs the whole call in
                        # the 1024-desc SWDGE ring -> device crash at 4480
                        single_packet=False,
                    )
                    msgs.append(m)

                for lb in range(G):
                    b = g * G + lb
                    pa = papool.tile([128, D], f32, tag="pa", name=f"pa{b}")
                    nmm = Q * Cq
                    i = 0
                    for q in range(Q):
                        for c in range(Cq):
                            j = lb * Cq + c
                            vh = vhpool.tile(
                                [128, 128], f32, tag="vh", name=f"vh{b}_{q}_{c}"
                            )
                            nc.vector.tensor_scalar(
                                vh[:],
                                iota_sb[:],
                                dst_t[:, q, j : j + 1],
                                val_t[:, q, j : j + 1],
                                eq,
                                mul,
                            )
                            nc.tensor.matmul(
                                pa[:],
                                vh[:],
                                msgs[q][:, j, :],
                                start=(i == 0),
                                stop=(i == nmm - 1),
                            )
                            i += 1
                    agg_sb = sbpool.tile([128, D], f32, tag="agg", name=f"agg{b}")
                    nc.vector.tensor_copy(agg_sb[:], pa[:])
                    pt = ptpool.tile([D, 128], f32, tag="pt", name=f"pt{b}")
                    nc.tensor.transpose(pt[:], agg_sb[:], id_sb[:])
                    aggT_sb = sbpool.tile([D, 128], f32, tag="aggT", name=f"aggT{b}")
                    nc.vector.tensor_copy(aggT_sb[:], pt[:])
                    po = popool.tile([D, 128], f32, tag="po", name=f"po{b}")
                    nc.tensor.matmul(po[:], w_sb[:], aggT_sb[:], start=True, stop=True)
                    nc.vector.tensor_copy(
                        outT_sb[:, b * 128 : (b + 1) * 128], po[:]
                    )

            nc.sync.dma_start(out=outT_d[:], in_=outT_sb[:])

    nc.compile()
    return nc


# ----------------------------------------------------------------- kernel()
def _ensure_ntff_hook():
    """Provide antenv.axon_hooks (absent in this image) so that
    run_bass_kernel_spmd's BASS_TRACE path can register the axon NTFF
    profiler instead of crashing on import."""
    try:
        import antenv.axon_hooks  # noqa: F401

        return
    except ImportError:
        pass
    import types

    import antenv

    mod = types.ModuleType("antenv.axon_hooks")
    holder = {"hook": None}
    mod.set_axon_ntff_profile_hook = lambda h: holder.__setitem__("hook", h)
    mod.get_axon_ntff_profile_hook = lambda: holder["hook"]
    sys.modules["antenv.axon_hooks"] = mod
    antenv.axon_hooks = mod
    try:
        from trn_agent_boot.trn_boot import _ntff_profile_via_ctypes

        mod.set_axon_ntff_profile_hook(
            _ntff_profile_via_ctypes("/opt/axon/libaxon_pjrt.so")
        )
    except Exception:
        pass


def kernel(x, weight, edge_vals, edge_row, edge_col):
    global LAST_EXEC_TIME_NS
    from concourse.bass_utils import run_bass_kernel_spmd

    if os.environ.get("BASS_TRACE"):
        _ensure_ntff_hook()

    in_maps, Cq = _prep(x, weight, edge_vals, edge_row, edge_col)
    if Cq not in _CACHE:
        _CACHE[Cq] = _build(Cq)
    nc = _CACHE[Cq]

    res = run_bass_kernel_spmd(nc, in_maps, list(range(CORES)))
    LAST_EXEC_TIME_NS = res.exec_time_ns

    outT = np.concatenate([res.results[k]["outT"] for k in range(CORES)], axis=1)
    out = np.ascontiguousarray(outT.T[:N])
    return out.astype(np.float32, copy=False)


# revision 5
# speedup vs baseline: 1.2254x; 1.2254x over previous
"""GCNConv (COO SpMM aggregation + dense GEMM) on 8 Trainium2 NeuronCores.

  msgs = edge_vals[:, None] * x[edge_col]          # [E, 64] gather+scale
  agg  = segment_sum(msgs, edge_row, N)            # [N, 64] scatter-add
  out  = agg @ weight                              # [N, 64] GEMM

Sharding: destination-node sharding (each core owns a contiguous row range
and all edges targeting it) -> zero collectives.  Host-side index prep sorts
edges by (core, 128-row dest block, x-quarter) and pads each (block, quarter)
group to a multiple of 128 edges.

Per core:
  - gpsimd.dma_gather pulls the 64-float source rows (256B each) from HBM
    (per-edge descriptors; desc-gen on the Q7s is the throughput limit)
  - one DVE tensor_tensor per (supergroup, quarter) scales msgs by edge_vals
    (broadcast AP along the feature dim)
  - one DVE tensor_tensor per dest block builds the one-hot
    oh[e, r] = (dest[e] == r) via iota-compare with a broadcast dest AP
  - TensorE per 128-edge chunk: psum[128 rows, 64] += oh.T @ msgs
    (segment-sum as matmul)
  - per 128-row block: PE-transpose agg -> aggT, then outT[64,128] = W.T@aggT
  - one contiguous [64, rows] output DMA; host concatenates + transposes.
"""

import os
import sys

import numpy as np

if "/opt/trn_rl_repo" not in sys.path:
    sys.path.insert(0, "/opt/trn_rl_repo")

# ---------------------------------------------------------------- constants
N = 100000
E = 1600000
D = 64
CORES = 8
RPC = 12544          # rows per core (= BLOCKS * 128; 8*12544 = 100352 >= N)
BLOCKS = RPC // 128  # 98 dest blocks per core
Q = 4                # x row-table quarters (int16 gather index limit)
XQ = 25088           # rows per quarter (4*25088 = 100352)
G = 2                # dest blocks per gather super-group (98 = 49*2)
NGROUPS = BLOCKS // G

LAST_EXEC_TIME_NS = None
_CACHE = {}


# ---------------------------------------------------------------- host prep
def _prep(x, weight, edge_vals, edge_row, edge_col):
    """Sort/pad edges; build per-core gather-index / dest / val arrays."""
    e_row = np.asarray(edge_row, dtype=np.int64)
    e_col = np.asarray(edge_col, dtype=np.int64)
    ev = np.asarray(edge_vals, dtype=np.float32)
    x = np.asarray(x, dtype=np.float32)
    weight = np.asarray(weight, dtype=np.float32)
    ne = e_row.shape[0]

    core = e_row // RPC
    blk = (e_row % RPC) // 128
    dest = (e_row % 128).astype(np.float32)
    qq = e_col // XQ
    lidx = (e_col - qq * XQ).astype(np.int16)

    NG = CORES * BLOCKS * Q
    gkey = (core * BLOCKS + blk) * Q + qq
    order = np.argsort(gkey, kind="stable")
    counts = np.bincount(gkey, minlength=NG)
    Cq = max(1, int(-(-counts.max() // 128)))  # chunks per (block, quarter)
    SL = Cq * 128
    starts = np.zeros(NG, np.int64)
    starts[1:] = np.cumsum(counts)[:-1]
    gs = gkey[order]
    slot = gs * SL + (np.arange(ne, dtype=np.int64) - starts[gs])

    idx_flat = np.zeros(NG * SL, np.int16)       # pad gathers row 0 (harmless)
    dst_flat = np.full(NG * SL, -1.0, np.float32)  # pad never matches iota
    val_flat = np.zeros(NG * SL, np.float32)       # pad scales to 0
    idx_flat[slot] = lidx[order]
    dst_flat[slot] = dest[order]
    val_flat[slot] = ev[order]

    CALLE = G * SL                   # edges per dma_gather call
    S16 = CALLE // 16

    def to_calls(a):
        # [CORES, BLOCKS, Q, SL] -> [CORES, NGROUPS, Q, G*SL] call-major
        a = a.reshape(CORES, NGROUPS, G, Q, SL)
        return np.ascontiguousarray(a.transpose(0, 1, 3, 2, 4)).reshape(
            CORES, NGROUPS, Q, CALLE
        )

    idx_c = to_calls(idx_flat)
    dst_c = to_calls(dst_flat)
    val_c = to_calls(val_flat)

    # gather idx wrap: idx for call-edge i lives at [i % 16, i // 16],
    # replicated across the 8 gpsimd cores -> [128, Q, CALLE//16]
    w16 = idx_c.reshape(CORES, NGROUPS, Q, S16, 16)
    w16 = np.moveaxis(w16, 4, 2)                      # [C, NGR, 16, Q, S16]
    gidx = np.ascontiguousarray(np.tile(w16, (1, 1, 8, 1, 1)))

    # dest/val layout matches gather output: [p, q, j] = call-edge j*128+p
    def to_pj(a):
        a = a.reshape(CORES, NGROUPS, Q, G * Cq, 128)
        return np.ascontiguousarray(np.moveaxis(a, 4, 2))  # [C,NGR,128,Q,G*Cq]

    gdst = to_pj(dst_c)
    gval = to_pj(val_c)

    x_pad = np.zeros((Q * XQ, D), np.float32)
    x_pad[:N] = x
    # iota replicated per chunk-slot: irep[p, s, m] = m
    irep = np.broadcast_to(
        np.arange(128, dtype=np.float32), (128, Q * Cq, 128)
    ).copy()
    ident = np.eye(128, dtype=np.float32)

    in_maps = []
    for k in range(CORES):
        in_maps.append(
            {
                "xq": x_pad,
                "w": np.ascontiguousarray(weight),
                "irep": irep,
                "ident": ident,
                "gidx": gidx[k],
                "gdst": gdst[k],
                "gval": gval[k],
            }
        )
    return in_maps, Cq


# ------------------------------------------------------------- bass program
def _build(Cq):
    import concourse.bacc as bacc
    import concourse.mybir as mybir
    import concourse.tile as tile

    f32 = mybir.dt.float32
    i16 = mybir.dt.int16
    SL = Cq * 128
    CALLE = G * SL
    S16 = CALLE // 16
    JQ = G * Cq          # msgs slots per (call, quarter)
    NCH = Q * Cq         # chunk-slots per block

    nc = bacc.Bacc(
        "TRN2", target_bir_lowering=False, debug=False, num_devices=CORES
    )
    x_d = nc.dram_tensor("xq", [Q * XQ, D], f32, kind="ExternalInput")
    w_d = nc.dram_tensor("w", [D, D], f32, kind="ExternalInput")
    irep_d = nc.dram_tensor("irep", [128, NCH, 128], f32, kind="ExternalInput")
    id_d = nc.dram_tensor("ident", [128, 128], f32, kind="ExternalInput")
    gidx_d = nc.dram_tensor("gidx", [NGROUPS, 128, Q, S16], i16, kind="ExternalInput")
    gdst_d = nc.dram_tensor("gdst", [NGROUPS, 128, Q, JQ], f32, kind="ExternalInput")
    gval_d = nc.dram_tensor("gval", [NGROUPS, 128, Q, JQ], f32, kind="ExternalInput")
    outT_d = nc.dram_tensor("outT", [D, RPC], f32, kind="ExternalOutput")

    eq = mybir.AluOpType.is_equal
    mul = mybir.AluOpType.mult

    with tile.TileContext(nc) as tc:
        with (
            tc.tile_pool(name="const", bufs=1) as cpool,
            tc.tile_pool(name="io", bufs=3) as iopool,
            tc.tile_pool(name="msgs", bufs=3) as mpool,
            tc.tile_pool(name="vh", bufs=3) as vhpool,
            tc.tile_pool(name="sb", bufs=4) as sbpool,
            tc.tile_pool(name="outsb", bufs=1) as opool,
            tc.tile_pool(name="pa", bufs=3, space="PSUM") as papool,
            tc.tile_pool(name="pt", bufs=2, space="PSUM") as ptpool,
            tc.tile_pool(name="po", bufs=2, space="PSUM") as popool,
        ):
            w_sb = cpool.tile([D, D], f32, name="w_sb")
            irep_sb = cpool.tile([128, NCH, 128], f32, name="irep_sb")
            id_sb = cpool.tile([128, 128], f32, name="id_sb")
            outT_sb = opool.tile([D, RPC], f32, name="outT_sb")
            nc.sync.dma_start(out=w_sb[:], in_=w_d[:])
            nc.sync.dma_start(out=irep_sb[:], in_=irep_d[:])
            nc.sync.dma_start(out=id_sb[:], in_=id_d[:])

            for g in range(NGROUPS):
                idx_t = iopool.tile([128, Q, S16], i16, tag="idx", name=f"idx{g}")
                dst_t = iopool.tile([128, Q, JQ], f32, tag="dst", name=f"dst{g}")
                val_t = iopool.tile([128, Q, JQ], f32, tag="val", name=f"val{g}")
                nc.sync.dma_start(out=idx_t[:], in_=gidx_d[g])
                nc.sync.dma_start(out=dst_t[:], in_=gdst_d[g])
                nc.sync.dma_start(out=val_t[:], in_=gval_d[g])

                msgs = []
                for q in range(Q):
                    m = mpool.tile([128, JQ, D], f32, tag=f"msgs{q}", name=f"m{g}_{q}")
                    nc.gpsimd.dma_gather(
                        m[:],
                        x_d[q * XQ : (q + 1) * XQ, :],
                        idx_t[:, q, :],
                        CALLE,
                        CALLE,
                        D,
                        # default single_packet=True needs the whole call in
                        # the 1024-desc SWDGE ring -> device crash at 4480
                        single_packet=False,
                    )
                    # scale msgs by edge_vals (broadcast along features)
                    nc.vector.tensor_tensor(
                        m[:],
                        m[:],
                        val_t[:, q, :].unsqueeze(2).broadcast_to([128, JQ, D]),
                        mul,
                    )
                    msgs.append(m)

                for lb in range(G):
                    b = g * G + lb
                    # one-hot for the whole block in one DVE op:
                    # vh[p, q, c, m] = (dest[p, q, lb*Cq+c] == m)
                    vh = vhpool.tile([128, Q, Cq, 128], f32, tag="vh", name=f"vh{b}")
                    nc.vector.tensor_tensor(
                        vh[:],
                        irep_sb[:].rearrange("p (q c) m -> p q c m", q=Q),
                        dst_t[:, :, lb * Cq : (lb + 1) * Cq]
                        .unsqueeze(3)
                        .broadcast_to([128, Q, Cq, 128]),
                        eq,
                    )
                    pa = papool.tile([128, D], f32, tag="pa", name=f"pa{b}")
                    nmm = Q * Cq
                    i = 0
                    for q in range(Q):
                        for c in range(Cq):
                            j = lb * Cq + c
                            nc.tensor.matmul(
                                pa[:],
                                vh[:, q, c, :],
                                msgs[q][:, j, :],
                                start=(i == 0),
                                stop=(i == nmm - 1),
                            )
                            i += 1
                    agg_sb = sbpool.tile([128, D], f32, tag="agg", name=f"agg{b}")
                    nc.vector.tensor_copy(agg_sb[:], pa[:])
                    pt = ptpool.tile([D, 128], f32, tag="pt", name=f"pt{b}")
                    nc.tensor.transpose(pt[:], agg_sb[:], id_sb[:])
                    aggT_sb = sbpool.tile([D, 128], f32, tag="aggT", name=f"aggT{b}")
                    nc.vector.tensor_copy(aggT_sb[:], pt[:])
                    po = popool.tile([D, 128], f32, tag="po", name=f"po{b}")
                    nc.tensor.matmul(po[:], w_sb[:], aggT_sb[:], start=True, stop=True)
                    nc.vector.tensor_copy(
                        outT_sb[:, b * 128 : (b + 1) * 128], po[:]
                    )

            nc.sync.dma_start(out=outT_d[:], in_=outT_sb[:])

    nc.compile()
    return nc


# ----------------------------------------------------------------- kernel()
def _ensure_ntff_hook():
    """Provide antenv.axon_hooks (absent in this image) so that
    run_bass_kernel_spmd's BASS_TRACE path can register the axon NTFF
    profiler instead of crashing on import."""
    try:
        import antenv.axon_hooks  # noqa: F401

        return
    except ImportError:
        pass
    import types

    import antenv

    mod = types.ModuleType("antenv.axon_hooks")
    holder = {"hook": None}
    mod.set_axon_ntff_profile_hook = lambda h: holder.__setitem__("hook", h)
    mod.get_axon_ntff_profile_hook = lambda: holder["hook"]
    sys.modules["antenv.axon_hooks"] = mod
    antenv.axon_hooks = mod
    try:
        from trn_agent_boot.trn_boot import _ntff_profile_via_ctypes

        mod.set_axon_ntff_profile_hook(
            _ntff_profile_via_ctypes("/opt/axon/libaxon_pjrt.so")
        )
    except Exception:
        pass


def kernel(x, weight, edge_vals, edge_row, edge_col):
    global LAST_EXEC_TIME_NS
    from concourse.bass_utils import run_bass_kernel_spmd

    if os.environ.get("BASS_TRACE"):
        _ensure_ntff_hook()

    in_maps, Cq = _prep(x, weight, edge_vals, edge_row, edge_col)
    if Cq not in _CACHE:
        _CACHE[Cq] = _build(Cq)
    nc = _CACHE[Cq]

    res = run_bass_kernel_spmd(nc, in_maps, list(range(CORES)))
    LAST_EXEC_TIME_NS = res.exec_time_ns

    outT = np.concatenate([res.results[k]["outT"] for k in range(CORES)], axis=1)
    out = np.ascontiguousarray(outT.T[:N])
    return out.astype(np.float32, copy=False)
